# revision 14
# baseline (speedup 1.0000x reference)
"""Causal attention (B=4, S=2048, D=1024, fp32) on 8 Trainium2 NeuronCores.

Sharding: data-parallel over batch (4) x query-split (2) per batch. The two
cores of a batch take interleaved query rows (even/odd within each 512-row
super-block), which makes the causal workload identical on every core and
lets one SPMD program serve all 8 cores; the only per-core differences are
pure data (which query columns of x^T each core receives, and the mask
tiles, which carry the even/odd offset).

Weight folding: scores = (x Wq)(x Wk)^T = x (Wq Wk^T) x^T, so the host
precomputes M = Wq Wk^T (a weight-only transform) and the device needs no
K projection at all: x^T itself is the key matrix, kept SBUF-resident, and
the only projections are q' = M^T-chunks applied to own queries and v.
The V projection chains are interleaved into the attention slot loop so the
PE never waits on a phase boundary.

All matmul inputs are bf16 (cast host-side; q'/v stored bf16 in SBUF; exp
probabilities bf16), accumulation in fp32 PSUM. Measured rel err ~5e-3 vs
the fp32 reference (gate 2e-2).

Attention per core:
  For each of 4 query slots s (256 queries from super-block [512s, 512s+512)):
    for key block kb in [0, 4s+4): scoresT = x_blk^T q'  -> +mask -> exp
      (no max-subtraction: scaled scores are ~N(0,1), exp is fp32-safe)
      denominators via ones-matmul; ctx accumulation in PSUM
    normalize by reciprocal(denom), DMA out.
"""

import numpy as np

B, S, D = 4, 2048, 1024
NE = D // 128          # contraction chunks (d on partitions)
NKBLK = S // 128       # 128-wide key blocks
NSLOT = 4              # query slots per core
QW = 256               # queries per slot
OWNQ = NSLOT * QW      # 1024 queries per core
MASK_NEG = -1.0e30
SCALE = 1.0 / 32.0     # 1/sqrt(D)

_cached = {}


def _build():
    import concourse.bacc as bacc
    import concourse.tile as tile
    import concourse.mybir as mybir
    from collections import deque

    F32 = mybir.dt.float32
    BF16 = mybir.dt.bfloat16
    EXP = mybir.ActivationFunctionType.Exp

    nc = bacc.Bacc("TRN2", target_bir_lowering=False, debug=False, num_devices=8,
                   dynamic_dma_scratch_size=2048)

    xt_d = nc.dram_tensor("xt", [D, S], BF16, kind="ExternalInput")
    xq_d = nc.dram_tensor("xq", [D, OWNQ], BF16, kind="ExternalInput")
    m_d = nc.dram_tensor("m", [D, D], BF16, kind="ExternalInput")
    wv_d = nc.dram_tensor("wv", [D, D], BF16, kind="ExternalInput")
    mask_d = nc.dram_tensor("masks", [128, 4 * QW], F32, kind="ExternalInput")
    ones_d = nc.dram_tensor("ones", [128, 2], BF16, kind="ExternalInput")
    o_d = nc.dram_tensor("o", [OWNQ, D], F32, kind="ExternalOutput")

    with tile.TileContext(nc) as tc:
        with (
            tc.tile_pool(name="res", bufs=1) as res,
            tc.tile_pool(name="ptp", bufs=4) as ptp,
            tc.tile_pool(name="obp", bufs=2) as obp,
            tc.tile_pool(name="rcp", bufs=2) as rcp,
            tc.tile_pool(name="rot", bufs=3, space="PSUM") as rot,
            tc.tile_pool(name="ctxp", bufs=1, space="PSUM") as ctxp,
            tc.tile_pool(name="dnp", bufs=1, space="PSUM") as dnp,
        ):
            # ---- resident tiles ----
            kx = []            # x^T chunks: the key matrix AND the V lhsT
            for c in range(NE):
                t = res.tile([128, S], BF16, name=f"kx{c}", tag=f"kx{c}")
                kx.append(t)
            vv = []
            for j in range(NKBLK):
                t = res.tile([128, D], BF16, name=f"v{j}", tag=f"v{j}")
                vv.append(t)
            qT = []
            for c in range(NE):
                t = res.tile([128, OWNQ], BF16, name=f"qT{c}", tag=f"qT{c}")
                qT.append(t)
            m_t = res.tile([128, NE * D], BF16, name="m_t", tag="m_t")
            wv_t = res.tile([128, NE * D], BF16, name="wv_t", tag="wv_t")
            xqr = res.tile([128, NE * OWNQ], BF16, name="xqr", tag="xqr")
            mask_t = res.tile([128, 4 * QW], F32, name="mask_t", tag="mask_t")
            ones_t = res.tile([128, 2], BF16, name="ones_t", tag="ones_t")

            # ---- input DMAs, in consumption order ----
            # V chains are the DMA-light filler work, so their inputs (wv +
            # kx) come first; m/xq (Q chains) and masks stream underneath
            # the V chains.
            for dc in range(NE):
                e1 = nc.sync if dc % 2 == 0 else nc.scalar
                e2 = nc.scalar if dc % 2 == 0 else nc.sync
                if dc < 2:
                    # Row-split the first chunks across 4 queues apiece (2KB
                    # lines preserved) so the first V chain starts ~4x sooner.
                    for r in range(4):
                        r0, r1 = r * 32, (r + 1) * 32
                        e1.dma_start(
                            wv_t[r0:r1, dc * D:(dc + 1) * D],
                            wv_d[dc * 128 + r0: dc * 128 + r1, :],
                        )
                        e2.dma_start(
                            kx[dc][r0:r1, 0:1024],
                            xt_d[dc * 128 + r0: dc * 128 + r1, 0:1024],
                        )
                else:
                    e1.dma_start(
                        wv_t[:, dc * D:(dc + 1) * D],
                        wv_d[dc * 128:(dc + 1) * 128, :],
                    )
                    e2.dma_start(kx[dc][:, 0:1024],
                                 xt_d[dc * 128:(dc + 1) * 128, 0:1024])
            for dc in range(NE):
                e1 = nc.sync if dc % 2 == 0 else nc.scalar
                e2 = nc.scalar if dc % 2 == 0 else nc.sync
                e1.dma_start(
                    m_t[:, dc * D:(dc + 1) * D], m_d[dc * 128:(dc + 1) * 128, :]
                )
                e2.dma_start(kx[dc][:, 1024:2048],
                             xt_d[dc * 128:(dc + 1) * 128, 1024:2048])
            nc.sync.dma_start(mask_t[:, :], mask_d[:, :])
            nc.sync.dma_start(ones_t[:, :], ones_d[:, :])
            for dc in range(NE):
                e = nc.sync if dc % 2 == 0 else nc.scalar
                e.dma_start(
                    xqr[:, dc * OWNQ:(dc + 1) * OWNQ],
                    xq_d[dc * 128:(dc + 1) * 128, :],
                )

            # ---- q' projection: qT[ei][:, jq*512 : +512] = sum_dc M-chunk^T xq ----
            def emit_q(jq):
                for ei in range(NE):
                    ps = rot.tile([128, 512], F32, name="rps", tag="rps")
                    for dc in range(NE):
                        nc.tensor.matmul(
                            ps[:, :],
                            m_t[:, dc * D + ei * 128: dc * D + (ei + 1) * 128],
                            xqr[:, dc * OWNQ + jq * 512: dc * OWNQ + (jq + 1) * 512],
                            start=(dc == 0), stop=(dc == NE - 1),
                        )
                    nc.scalar.copy(qT[ei][:, jq * 512:(jq + 1) * 512], ps[:, :])

            # ---- V chain emitter (interleaved into the attention loop) ----
            vq = deque(range(NKBLK * 2))  # (jc, dh) halves in jc-major order

            def emit_v(n):
                while n > 0 and vq:
                    idx = vq.popleft()
                    jc, dh = idx // 2, idx % 2
                    ps = rot.tile([128, 512], F32, name="rps", tag="rps")
                    for dc in range(NE):
                        nc.tensor.matmul(
                            ps[:, :],
                            kx[dc][:, jc * 128:(jc + 1) * 128],
                            wv_t[:, dc * D + dh * 512: dc * D + (dh + 1) * 512],
                            start=(dc == 0), stop=(dc == NE - 1),
                        )
                    nc.vector.tensor_copy(vv[jc][:, dh * 512:(dh + 1) * 512], ps[:, :])
                    n -= 1

            # ---- interleave projections so early compute needs early bytes ----
            # V chains jc 0-7 fill the PE while m/xq stream in; Q chains run
            # once their inputs land; remaining V weaves into the slot loop.
            emit_v(16)     # jc 0..7
            emit_q(0)
            emit_v(8)      # jc 8..11
            emitted0 = 24

            # ---- attention, V chains woven between score blocks ----
            def consume(item):
                s, kb, pt, ctx, dn = item
                nk = 4 * s + 4
                for c in range(2):
                    # Both column groups live in one PSUM bank; start=True
                    # clears the whole bank, so only the first group may
                    # set it — the second lands on freshly cleared psum
                    # (has_written=0) and still overwrites, not adds.
                    nc.tensor.matmul(
                        dn[:, 2 * c:2 * c + 2],
                        pt[:, c * 128:(c + 1) * 128],
                        ones_t[:, :],
                        start=(kb == 0 and c == 0), stop=(kb == nk - 1),
                        skip_group_check=True,
                    )
                    for dh in range(2):
                        nc.tensor.matmul(
                            ctx[(c, dh)][:, :],
                            pt[:, c * 128:(c + 1) * 128],
                            vv[kb][:, dh * 512:(dh + 1) * 512],
                            start=(kb == 0), stop=(kb == nk - 1),
                        )
                if kb == nk - 1:
                    rc = rcp.tile([128, 2], F32, name="rc", tag="rc")
                    nc.vector.reciprocal(rc[:, :], dn[:, 0:4:2])
                    for c in range(2):
                        ob = obp.tile([128, D], F32, name="ob", tag="ob")
                        for dh in range(2):
                            nc.vector.tensor_scalar_mul(
                                ob[:, dh * 512:(dh + 1) * 512],
                                ctx[(c, dh)][:, :],
                                rc[:, c:c + 1],
                            )
                            # fire each scaled half as 2 row-split DMAs so the
                            # final slot's writeback spreads over 4 queues
                            for r in range(2):
                                r0, r1 = r * 64, (r + 1) * 64
                                e = nc.sync if (dh + r) % 2 == 0 else nc.scalar
                                e.dma_start(
                                    o_d[s * QW + c * 128 + r0:
                                        s * QW + c * 128 + r1,
                                        dh * 512:(dh + 1) * 512],
                                    ob[r0:r1, dh * 512:(dh + 1) * 512],
                                )

            pending = deque()
            DEPTH = 2
            emitted = emitted0
            for s in range(NSLOT):
                nk = 4 * s + 4
                if s == 1:
                    # qT's second half is only read from slot 2 on; emitting
                    # its chains here keeps them clear of slot 0's start
                    # while xqr finishes streaming in.
                    emit_q(1)
                # vv[0..nk-1] writes must be emitted before this slot's ctx
                # matmuls reference them (Tile deps follow emission order).
                need = 2 * nk
                if emitted < need:
                    emit_v(need - emitted)
                    emitted = need
                while pending:
                    consume(pending.popleft())
                ctx_cur = {}
                for c in range(2):
                    for dh in range(2):
                        t = ctxp.tile(
                            [128, 512], F32,
                            name=f"ctx{c}{dh}", tag=f"ctx{c}{dh}",
                        )
                        ctx_cur[(c, dh)] = t
                dn_cur = dnp.tile([128, 4], F32, name="dn", tag="dn")
                for kb in range(nk):
                    ps_sc = rot.tile([128, 512], F32, name="rps", tag="rps")
                    for ec in range(NE):
                        nc.tensor.matmul(
                            ps_sc[:, 0:QW],
                            kx[ec][:, kb * 128:(kb + 1) * 128],
                            qT[ec][:, s * QW:(s + 1) * QW],
                            start=(ec == 0), stop=(ec == NE - 1),
                        )
                    t_idx = kb - (nk - 4)
                    if t_idx >= 0:
                        nc.vector.tensor_add(
                            ps_sc[:, 0:QW], ps_sc[:, 0:QW],
                            mask_t[:, t_idx * QW:(t_idx + 1) * QW],
                        )
                    pt = ptp.tile([128, QW], BF16, name="pt", tag="pt")
                    nc.scalar.activation(pt[:, :], ps_sc[:, 0:QW], EXP, scale=SCALE)
                    pending.append((s, kb, pt, ctx_cur, dn_cur))
                    if len(pending) > DEPTH:
                        consume(pending.popleft())
                    # weave one V chain between score blocks while any remain
                    if vq:
                        emit_v(1)
                        emitted += 1
            while pending:
                consume(pending.popleft())

    nc.compile()
    return nc


def _get_nc():
    if "nc" not in _cached:
        _cached["nc"] = _build()
    return _cached["nc"]


def build_in_maps(x, W_q, W_k, W_v):
    import ml_dtypes

    BF = ml_dtypes.bfloat16
    x = np.asarray(x, dtype=np.float32)
    wq = np.asarray(W_q, dtype=np.float32)
    wk = np.asarray(W_k, dtype=np.float32)
    m = np.ascontiguousarray(wq @ wk.T).astype(BF)
    wv = np.ascontiguousarray(np.asarray(W_v, dtype=np.float32).astype(BF))
    ones = np.ones((128, 2), dtype=BF)

    p = np.arange(128, dtype=np.int64)[:, None]
    f = np.arange(QW, dtype=np.int64)[None, :]
    masks_h = []
    for h in range(2):
        tiles = [
            np.where(128 * t + p <= 2 * f + h, np.float32(0.0), np.float32(MASK_NEG))
            for t in range(4)
        ]
        masks_h.append(np.concatenate(tiles, axis=1).astype(np.float32))

    xbf = x.astype(BF)
    xt_b = [np.ascontiguousarray(xbf[b].T) for b in range(B)]
    in_maps = []
    for c in range(8):
        b, h = c // 2, c % 2
        xq = np.ascontiguousarray(xbf[b, h::2, :].T)
        in_maps.append({
            "xt": xt_b[b],
            "xq": xq,
            "m": m,
            "wv": wv,
            "masks": masks_h[h],
            "ones": ones,
        })
    return in_maps


def kernel(x, W_q, W_k, W_v):
    from concourse.bass_utils import run_bass_kernel_spmd

    in_maps = build_in_maps(x, W_q, W_k, W_v)
    nc = _get_nc()
    res = run_bass_kernel_spmd(nc, in_maps, core_ids=list(range(8)))

    out = np.empty((B, S, D), dtype=np.float32)
    for c in range(8):
        b, h = c // 2, c % 2
        out[b, h::2, :] = res.results[c]["o"]
    return out


# revision 16
# speedup vs baseline: 1.0152x; 1.0152x over previous
"""Causal attention (B=4, S=2048, D=1024, fp32) on 8 Trainium2 NeuronCores.

Sharding: data-parallel over batch (4) x query-split (2) per batch. The two
cores of a batch take interleaved query rows (even/odd within each 512-row
super-block), which makes the causal workload identical on every core and
lets one SPMD program serve all 8 cores; the only per-core differences are
pure data (which query columns of x^T each core receives, and the mask
tiles, which carry the even/odd offset).

Weight folding: scores = (x Wq)(x Wk)^T = x (Wq Wk^T) x^T, so the host
precomputes M = Wq Wk^T (a weight-only transform) and the device needs no
K projection at all: x^T itself is the key matrix, kept SBUF-resident, and
the only projections are q' = M^T-chunks applied to own queries and v.
The V projection chains are interleaved into the attention slot loop so the
PE never waits on a phase boundary.

All matmul inputs are bf16 (cast host-side; q'/v stored bf16 in SBUF; exp
probabilities bf16), accumulation in fp32 PSUM. Measured rel err ~5e-3 vs
the fp32 reference (gate 2e-2).

Attention per core:
  For each of 4 query slots s (256 queries from super-block [512s, 512s+512)):
    for key block kb in [0, 4s+4): scoresT = x_blk^T q'  -> +mask -> exp
      (no max-subtraction: scaled scores are ~N(0,1), exp is fp32-safe)
      denominators via ones-matmul; ctx accumulation in PSUM
    normalize by reciprocal(denom), DMA out.
"""

import numpy as np

B, S, D = 4, 2048, 1024
NE = D // 128          # contraction chunks (d on partitions)
NKBLK = S // 128       # 128-wide key blocks
NSLOT = 4              # query slots per core
QW = 256               # queries per slot
OWNQ = NSLOT * QW      # 1024 queries per core
MASK_NEG = -1.0e30
SCALE = 1.0 / 32.0     # 1/sqrt(D)

_cached = {}


def _build():
    import concourse.bacc as bacc
    import concourse.tile as tile
    import concourse.mybir as mybir
    from collections import deque

    F32 = mybir.dt.float32
    BF16 = mybir.dt.bfloat16
    EXP = mybir.ActivationFunctionType.Exp

    nc = bacc.Bacc("TRN2", target_bir_lowering=False, debug=False, num_devices=8,
                   dynamic_dma_scratch_size=2048)

    xt_d = nc.dram_tensor("xt", [D, S], BF16, kind="ExternalInput")
    xq_d = nc.dram_tensor("xq", [D, OWNQ], BF16, kind="ExternalInput")
    m_d = nc.dram_tensor("m", [D, D], BF16, kind="ExternalInput")
    wv_d = nc.dram_tensor("wv", [D, D], BF16, kind="ExternalInput")
    mask_d = nc.dram_tensor("masks", [128, 4 * QW], F32, kind="ExternalInput")
    ones_d = nc.dram_tensor("ones", [128, 2], BF16, kind="ExternalInput")
    o_d = nc.dram_tensor("o", [OWNQ, D], F32, kind="ExternalOutput")

    with tile.TileContext(nc) as tc:
        with (
            tc.tile_pool(name="res", bufs=1) as res,
            tc.tile_pool(name="ptp", bufs=4) as ptp,
            tc.tile_pool(name="obp", bufs=2) as obp,
            tc.tile_pool(name="rcp", bufs=2) as rcp,
            tc.tile_pool(name="rot", bufs=3, space="PSUM") as rot,
            tc.tile_pool(name="ctxp", bufs=1, space="PSUM") as ctxp,
            tc.tile_pool(name="dnp", bufs=1, space="PSUM") as dnp,
        ):
            # ---- resident tiles ----
            kx = []            # x^T chunks: the key matrix AND the V lhsT
            for c in range(NE):
                t = res.tile([128, S], BF16, name=f"kx{c}", tag=f"kx{c}")
                kx.append(t)
            vv = []
            for j in range(NKBLK):
                t = res.tile([128, D], BF16, name=f"v{j}", tag=f"v{j}")
                vv.append(t)
            qT = []
            for c in range(NE):
                t = res.tile([128, OWNQ], BF16, name=f"qT{c}", tag=f"qT{c}")
                qT.append(t)
            m_t = res.tile([128, NE * D], BF16, name="m_t", tag="m_t")
            wv_t = res.tile([128, NE * D], BF16, name="wv_t", tag="wv_t")
            xqr = res.tile([128, NE * OWNQ], BF16, name="xqr", tag="xqr")
            mask_t = res.tile([128, 4 * QW], F32, name="mask_t", tag="mask_t")
            ones_t = res.tile([128, 2], BF16, name="ones_t", tag="ones_t")

            # ---- input DMAs, in consumption order ----
            # V chains are the DMA-light filler work, so their inputs (wv +
            # kx) come first; m/xq (Q chains) and masks stream underneath
            # the V chains.
            for dc in range(NE):
                e1 = nc.sync if dc % 2 == 0 else nc.scalar
                e2 = nc.scalar if dc % 2 == 0 else nc.sync
                e1.dma_start(
                    wv_t[:, dc * D:(dc + 1) * D], wv_d[dc * 128:(dc + 1) * 128, :]
                )
                e2.dma_start(kx[dc][:, :], xt_d[dc * 128:(dc + 1) * 128, :])
            for dc in range(NE):
                e = nc.sync if dc % 2 == 0 else nc.scalar
                e.dma_start(
                    m_t[:, dc * D:(dc + 1) * D], m_d[dc * 128:(dc + 1) * 128, :]
                )
            nc.sync.dma_start(mask_t[:, :], mask_d[:, :])
            nc.sync.dma_start(ones_t[:, :], ones_d[:, :])
            for dc in range(NE):
                e = nc.sync if dc % 2 == 0 else nc.scalar
                e.dma_start(
                    xqr[:, dc * OWNQ:(dc + 1) * OWNQ],
                    xq_d[dc * 128:(dc + 1) * 128, :],
                )

            # ---- q' projection: qT[ei][:, jq*512 : +512] = sum_dc M-chunk^T xq ----
            def emit_q(jq):
                for ei in range(NE):
                    ps = rot.tile([128, 512], F32, name="rps", tag="rps")
                    for dc in range(NE):
                        nc.tensor.matmul(
                            ps[:, :],
                            m_t[:, dc * D + ei * 128: dc * D + (ei + 1) * 128],
                            xqr[:, dc * OWNQ + jq * 512: dc * OWNQ + (jq + 1) * 512],
                            start=(dc == 0), stop=(dc == NE - 1),
                        )
                    nc.scalar.copy(qT[ei][:, jq * 512:(jq + 1) * 512], ps[:, :])

            # ---- V chain emitter (interleaved into the attention loop) ----
            vq = deque(range(NKBLK * 2))  # (jc, dh) halves in jc-major order

            def emit_v(n):
                while n > 0 and vq:
                    idx = vq.popleft()
                    jc, dh = idx // 2, idx % 2
                    ps = rot.tile([128, 512], F32, name="rps", tag="rps")
                    for dc in range(NE):
                        nc.tensor.matmul(
                            ps[:, :],
                            kx[dc][:, jc * 128:(jc + 1) * 128],
                            wv_t[:, dc * D + dh * 512: dc * D + (dh + 1) * 512],
                            start=(dc == 0), stop=(dc == NE - 1),
                        )
                    nc.vector.tensor_copy(vv[jc][:, dh * 512:(dh + 1) * 512], ps[:, :])
                    n -= 1

            # ---- interleave projections so early compute needs early bytes ----
            # V chains jc 0-7 fill the PE while m/xq stream in; Q chains run
            # once their inputs land; remaining V weaves into the slot loop.
            emit_v(16)     # jc 0..7
            emit_q(0)
            emit_v(8)      # jc 8..11
            emitted0 = 24

            # ---- attention, V chains woven between score blocks ----
            def consume(item):
                s, kb, pt, ctx, dn = item
                nk = 4 * s + 4
                for c in range(2):
                    # Both column groups live in one PSUM bank; start=True
                    # clears the whole bank, so only the first group may
                    # set it — the second lands on freshly cleared psum
                    # (has_written=0) and still overwrites, not adds.
                    nc.tensor.matmul(
                        dn[:, 2 * c:2 * c + 2],
                        pt[:, c * 128:(c + 1) * 128],
                        ones_t[:, :],
                        start=(kb == 0 and c == 0), stop=(kb == nk - 1),
                        skip_group_check=True,
                    )
                    for dh in range(2):
                        nc.tensor.matmul(
                            ctx[(c, dh)][:, :],
                            pt[:, c * 128:(c + 1) * 128],
                            vv[kb][:, dh * 512:(dh + 1) * 512],
                            start=(kb == 0), stop=(kb == nk - 1),
                        )
                if kb == nk - 1:
                    rc = rcp.tile([128, 2], F32, name="rc", tag="rc")
                    nc.vector.reciprocal(rc[:, :], dn[:, 0:4:2])
                    for c in range(2):
                        ob = obp.tile([128, D], F32, name="ob", tag="ob")
                        for dh in range(2):
                            nc.vector.tensor_scalar_mul(
                                ob[:, dh * 512:(dh + 1) * 512],
                                ctx[(c, dh)][:, :],
                                rc[:, c:c + 1],
                            )
                        e = nc.sync if c == 0 else nc.scalar
                        e.dma_start(
                            o_d[s * QW + c * 128: s * QW + (c + 1) * 128, :],
                            ob[:, :],
                        )

            pending = deque()
            DEPTH = 2
            emitted = emitted0
            for s in range(NSLOT):
                nk = 4 * s + 4
                if s == 1:
                    # qT's second half is only read from slot 2 on; emitting
                    # its chains here keeps them clear of slot 0's start
                    # while xqr finishes streaming in.
                    emit_q(1)
                # vv[0..nk-1] writes must be emitted before this slot's ctx
                # matmuls reference them (Tile deps follow emission order).
                need = 2 * nk
                if emitted < need:
                    emit_v(need - emitted)
                    emitted = need
                while pending:
                    consume(pending.popleft())
                ctx_cur = {}
                for c in range(2):
                    for dh in range(2):
                        t = ctxp.tile(
                            [128, 512], F32,
                            name=f"ctx{c}{dh}", tag=f"ctx{c}{dh}",
                        )
                        ctx_cur[(c, dh)] = t
                dn_cur = dnp.tile([128, 4], F32, name="dn", tag="dn")
                for kb in range(nk):
                    ps_sc = rot.tile([128, 512], F32, name="rps", tag="rps")
                    for ec in range(NE):
                        nc.tensor.matmul(
                            ps_sc[:, 0:QW],
                            kx[ec][:, kb * 128:(kb + 1) * 128],
                            qT[ec][:, s * QW:(s + 1) * QW],
                            start=(ec == 0), stop=(ec == NE - 1),
                        )
                    t_idx = kb - (nk - 4)
                    if t_idx >= 0:
                        nc.vector.tensor_add(
                            ps_sc[:, 0:QW], ps_sc[:, 0:QW],
                            mask_t[:, t_idx * QW:(t_idx + 1) * QW],
                        )
                    pt = ptp.tile([128, QW], BF16, name="pt", tag="pt")
                    nc.scalar.activation(pt[:, :], ps_sc[:, 0:QW], EXP, scale=SCALE)
                    pending.append((s, kb, pt, ctx_cur, dn_cur))
                    if len(pending) > DEPTH:
                        consume(pending.popleft())
                    # weave one V chain between score blocks while any remain
                    if vq:
                        emit_v(1)
                        emitted += 1
            while pending:
                consume(pending.popleft())

    nc.compile()
    return nc


def _get_nc():
    if "nc" not in _cached:
        _cached["nc"] = _build()
    return _cached["nc"]


def build_in_maps(x, W_q, W_k, W_v):
    import ml_dtypes

    BF = ml_dtypes.bfloat16
    x = np.asarray(x, dtype=np.float32)
    wq = np.asarray(W_q, dtype=np.float32)
    wk = np.asarray(W_k, dtype=np.float32)
    m = np.ascontiguousarray(wq @ wk.T).astype(BF)
    wv = np.ascontiguousarray(np.asarray(W_v, dtype=np.float32).astype(BF))
    ones = np.ones((128, 2), dtype=BF)

    p = np.arange(128, dtype=np.int64)[:, None]
    f = np.arange(QW, dtype=np.int64)[None, :]
    masks_h = []
    for h in range(2):
        tiles = [
            np.where(128 * t + p <= 2 * f + h, np.float32(0.0), np.float32(MASK_NEG))
            for t in range(4)
        ]
        masks_h.append(np.concatenate(tiles, axis=1).astype(np.float32))

    xbf = x.astype(BF)
    xt_b = [np.ascontiguousarray(xbf[b].T) for b in range(B)]
    in_maps = []
    for c in range(8):
        b, h = c // 2, c % 2
        xq = np.ascontiguousarray(xbf[b, h::2, :].T)
        in_maps.append({
            "xt": xt_b[b],
            "xq": xq,
            "m": m,
            "wv": wv,
            "masks": masks_h[h],
            "ones": ones,
        })
    return in_maps


def kernel(x, W_q, W_k, W_v):
    from concourse.bass_utils import run_bass_kernel_spmd

    in_maps = build_in_maps(x, W_q, W_k, W_v)
    nc = _get_nc()
    res = run_bass_kernel_spmd(nc, in_maps, core_ids=list(range(8)))

    out = np.empty((B, S, D), dtype=np.float32)
    for c in range(8):
        b, h = c // 2, c % 2
        out[b, h::2, :] = res.results[c]["o"]
    return out


# revision 18
# speedup vs baseline: 1.0480x; 1.0323x over previous
"""Causal attention (B=4, S=2048, D=1024, fp32) on 8 Trainium2 NeuronCores.

Sharding: data-parallel over batch (4) x query-split (2) per batch. The two
cores of a batch take interleaved query rows (even/odd within each 512-row
super-block), which makes the causal workload identical on every core and
lets one SPMD program serve all 8 cores; the only per-core differences are
pure data (which query columns of x^T each core receives, and the mask
tiles, which carry the even/odd offset).

Weight folding: scores = (x Wq)(x Wk)^T = x (Wq Wk^T) x^T, so the host
precomputes M = Wq Wk^T (a weight-only transform) and the device needs no
K projection at all: x^T itself is the key matrix, kept SBUF-resident, and
the only projections are q' = M^T-chunks applied to own queries and v.
The V projection chains are interleaved into the attention slot loop so the
PE never waits on a phase boundary.

All matmul inputs are bf16 (cast host-side; q'/v stored bf16 in SBUF; exp
probabilities bf16), accumulation in fp32 PSUM. Measured rel err ~5e-3 vs
the fp32 reference (gate 2e-2).

Attention per core:
  For each of 4 query slots s (256 queries from super-block [512s, 512s+512)):
    for key block kb in [0, 4s+4): scoresT = x_blk^T q'  -> +mask -> exp
      (no max-subtraction: scaled scores are ~N(0,1), exp is fp32-safe)
      denominators via ones-matmul; ctx accumulation in PSUM
    normalize by reciprocal(denom), DMA out.
"""

import numpy as np

B, S, D = 4, 2048, 1024
NE = D // 128          # contraction chunks (d on partitions)
NKBLK = S // 128       # 128-wide key blocks
NSLOT = 4              # query slots per core
QW = 256               # queries per slot
OWNQ = NSLOT * QW      # 1024 queries per core
MASK_NEG = -1.0e30
SCALE = 1.0 / 32.0     # 1/sqrt(D)

_cached = {}


def _build():
    import concourse.bacc as bacc
    import concourse.tile as tile
    import concourse.mybir as mybir
    from collections import deque

    F32 = mybir.dt.float32
    BF16 = mybir.dt.bfloat16
    EXP = mybir.ActivationFunctionType.Exp

    nc = bacc.Bacc("TRN2", target_bir_lowering=False, debug=False, num_devices=8,
                   dynamic_dma_scratch_size=2048)

    xt_d = nc.dram_tensor("xt", [D, S], BF16, kind="ExternalInput")
    xq_d = nc.dram_tensor("xq", [D, OWNQ], BF16, kind="ExternalInput")
    m_d = nc.dram_tensor("m", [D, D], BF16, kind="ExternalInput")
    wv_d = nc.dram_tensor("wv", [D, D], BF16, kind="ExternalInput")
    mask_d = nc.dram_tensor("masks", [128, 4 * QW], F32, kind="ExternalInput")
    ones_d = nc.dram_tensor("ones", [128, 2], BF16, kind="ExternalInput")
    o_d = nc.dram_tensor("o", [OWNQ, D], F32, kind="ExternalOutput")

    with tile.TileContext(nc) as tc:
        with (
            tc.tile_pool(name="res", bufs=1) as res,
            tc.tile_pool(name="ptp", bufs=4) as ptp,
            tc.tile_pool(name="obp", bufs=2) as obp,
            tc.tile_pool(name="rcp", bufs=2) as rcp,
            tc.tile_pool(name="rot", bufs=3, space="PSUM") as rot,
            tc.tile_pool(name="ctxp", bufs=1, space="PSUM") as ctxp,
            tc.tile_pool(name="dnp", bufs=1, space="PSUM") as dnp,
        ):
            # ---- resident tiles ----
            kx = []            # x^T chunks: the key matrix AND the V lhsT
            for c in range(NE):
                t = res.tile([128, S], BF16, name=f"kx{c}", tag=f"kx{c}")
                kx.append(t)
            vv = []
            for j in range(NKBLK):
                t = res.tile([128, D], BF16, name=f"v{j}", tag=f"v{j}")
                vv.append(t)
            qT = []
            for c in range(NE):
                t = res.tile([128, OWNQ], BF16, name=f"qT{c}", tag=f"qT{c}")
                qT.append(t)
            m_t = res.tile([128, NE * D], BF16, name="m_t", tag="m_t")
            wv_t = res.tile([128, NE * D], BF16, name="wv_t", tag="wv_t")
            xqr = res.tile([128, NE * OWNQ], BF16, name="xqr", tag="xqr")
            mask_t = res.tile([128, 4 * QW], F32, name="mask_t", tag="mask_t")
            ones_t = res.tile([128, 2], BF16, name="ones_t", tag="ones_t")

            # ---- input DMAs, in consumption order ----
            # V chains are the DMA-light filler work, so their inputs (wv +
            # kx) come first; m/xq (Q chains) and masks stream underneath
            # the V chains.
            for dc in range(NE):
                e1 = nc.sync if dc % 2 == 0 else nc.scalar
                e2 = nc.scalar if dc % 2 == 0 else nc.sync
                e1.dma_start(
                    wv_t[:, dc * D:(dc + 1) * D], wv_d[dc * 128:(dc + 1) * 128, :]
                )
                e2.dma_start(kx[dc][:, 0:1024],
                             xt_d[dc * 128:(dc + 1) * 128, 0:1024])
            for dc in range(NE):
                e1 = nc.sync if dc % 2 == 0 else nc.scalar
                e2 = nc.scalar if dc % 2 == 0 else nc.sync
                e1.dma_start(
                    m_t[:, dc * D:(dc + 1) * D], m_d[dc * 128:(dc + 1) * 128, :]
                )
                e2.dma_start(kx[dc][:, 1024:2048],
                             xt_d[dc * 128:(dc + 1) * 128, 1024:2048])
            nc.sync.dma_start(mask_t[:, :], mask_d[:, :])
            nc.sync.dma_start(ones_t[:, :], ones_d[:, :])
            for dc in range(NE):
                e = nc.sync if dc % 2 == 0 else nc.scalar
                e.dma_start(
                    xqr[:, dc * OWNQ:(dc + 1) * OWNQ],
                    xq_d[dc * 128:(dc + 1) * 128, :],
                )

            # ---- q' projection: qT[ei][:, jq*512 : +512] = sum_dc M-chunk^T xq ----
            def emit_q(jq):
                for ei in range(NE):
                    ps = rot.tile([128, 512], F32, name="rps", tag="rps")
                    for dc in range(NE):
                        nc.tensor.matmul(
                            ps[:, :],
                            m_t[:, dc * D + ei * 128: dc * D + (ei + 1) * 128],
                            xqr[:, dc * OWNQ + jq * 512: dc * OWNQ + (jq + 1) * 512],
                            start=(dc == 0), stop=(dc == NE - 1),
                        )
                    nc.scalar.copy(qT[ei][:, jq * 512:(jq + 1) * 512], ps[:, :])

            # ---- V chain emitter (interleaved into the attention loop) ----
            vq = deque(range(NKBLK * 2))  # (jc, dh) halves in jc-major order

            def emit_v(n):
                while n > 0 and vq:
                    idx = vq.popleft()
                    jc, dh = idx // 2, idx % 2
                    ps = rot.tile([128, 512], F32, name="rps", tag="rps")
                    for dc in range(NE):
                        nc.tensor.matmul(
                            ps[:, :],
                            kx[dc][:, jc * 128:(jc + 1) * 128],
                            wv_t[:, dc * D + dh * 512: dc * D + (dh + 1) * 512],
                            start=(dc == 0), stop=(dc == NE - 1),
                        )
                    nc.vector.tensor_copy(vv[jc][:, dh * 512:(dh + 1) * 512], ps[:, :])
                    n -= 1

            # ---- interleave projections so early compute needs early bytes ----
            # V chains jc 0-7 fill the PE while m/xq stream in; Q chains run
            # once their inputs land; remaining V weaves into the slot loop.
            emit_v(16)     # jc 0..7
            emit_q(0)
            emit_v(8)      # jc 8..11
            emitted0 = 24

            # ---- attention, V chains woven between score blocks ----
            def consume(item):
                s, kb, pt, ctx, dn = item
                nk = 4 * s + 4
                for c in range(2):
                    # Both column groups live in one PSUM bank; start=True
                    # clears the whole bank, so only the first group may
                    # set it — the second lands on freshly cleared psum
                    # (has_written=0) and still overwrites, not adds.
                    nc.tensor.matmul(
                        dn[:, 2 * c:2 * c + 2],
                        pt[:, c * 128:(c + 1) * 128],
                        ones_t[:, :],
                        start=(kb == 0 and c == 0), stop=(kb == nk - 1),
                        skip_group_check=True,
                    )
                    for dh in range(2):
                        nc.tensor.matmul(
                            ctx[(c, dh)][:, :],
                            pt[:, c * 128:(c + 1) * 128],
                            vv[kb][:, dh * 512:(dh + 1) * 512],
                            start=(kb == 0), stop=(kb == nk - 1),
                        )
                if kb == nk - 1:
                    rc = rcp.tile([128, 2], F32, name="rc", tag="rc")
                    nc.vector.reciprocal(rc[:, :], dn[:, 0:4:2])
                    for c in range(2):
                        ob = obp.tile([128, D], F32, name="ob", tag="ob")
                        for dh in range(2):
                            nc.vector.tensor_scalar_mul(
                                ob[:, dh * 512:(dh + 1) * 512],
                                ctx[(c, dh)][:, :],
                                rc[:, c:c + 1],
                            )
                            # fire each 512-col half as soon as it's scaled
                            e = nc.sync if dh == 0 else nc.scalar
                            e.dma_start(
                                o_d[s * QW + c * 128: s * QW + (c + 1) * 128,
                                    dh * 512:(dh + 1) * 512],
                                ob[:, dh * 512:(dh + 1) * 512],
                            )

            pending = deque()
            DEPTH = 2
            emitted = emitted0
            for s in range(NSLOT):
                nk = 4 * s + 4
                if s == 1:
                    # qT's second half is only read from slot 2 on; emitting
                    # its chains here keeps them clear of slot 0's start
                    # while xqr finishes streaming in.
                    emit_q(1)
                # vv[0..nk-1] writes must be emitted before this slot's ctx
                # matmuls reference them (Tile deps follow emission order).
                need = 2 * nk
                if emitted < need:
                    emit_v(need - emitted)
                    emitted = need
                while pending:
                    consume(pending.popleft())
                ctx_cur = {}
                for c in range(2):
                    for dh in range(2):
                        t = ctxp.tile(
                            [128, 512], F32,
                            name=f"ctx{c}{dh}", tag=f"ctx{c}{dh}",
                        )
                        ctx_cur[(c, dh)] = t
                dn_cur = dnp.tile([128, 4], F32, name="dn", tag="dn")
                for kb in range(nk):
                    ps_sc = rot.tile([128, 512], F32, name="rps", tag="rps")
                    for ec in range(NE):
                        nc.tensor.matmul(
                            ps_sc[:, 0:QW],
                            kx[ec][:, kb * 128:(kb + 1) * 128],
                            qT[ec][:, s * QW:(s + 1) * QW],
                            start=(ec == 0), stop=(ec == NE - 1),
                        )
                    t_idx = kb - (nk - 4)
                    if t_idx >= 0:
                        nc.vector.tensor_add(
                            ps_sc[:, 0:QW], ps_sc[:, 0:QW],
                            mask_t[:, t_idx * QW:(t_idx + 1) * QW],
                        )
                    pt = ptp.tile([128, QW], BF16, name="pt", tag="pt")
                    nc.scalar.activation(pt[:, :], ps_sc[:, 0:QW], EXP, scale=SCALE)
                    pending.append((s, kb, pt, ctx_cur, dn_cur))
                    if len(pending) > DEPTH:
                        consume(pending.popleft())
                    # weave one V chain between score blocks while any remain
                    if vq:
                        emit_v(1)
                        emitted += 1
            while pending:
                consume(pending.popleft())

    nc.compile()
    return nc


def _get_nc():
    if "nc" not in _cached:
        _cached["nc"] = _build()
    return _cached["nc"]


def build_in_maps(x, W_q, W_k, W_v):
    import ml_dtypes

    BF = ml_dtypes.bfloat16
    x = np.asarray(x, dtype=np.float32)
    wq = np.asarray(W_q, dtype=np.float32)
    wk = np.asarray(W_k, dtype=np.float32)
    m = np.ascontiguousarray(wq @ wk.T).astype(BF)
    wv = np.ascontiguousarray(np.asarray(W_v, dtype=np.float32).astype(BF))
    ones = np.ones((128, 2), dtype=BF)

    p = np.arange(128, dtype=np.int64)[:, None]
    f = np.arange(QW, dtype=np.int64)[None, :]
    masks_h = []
    for h in range(2):
        tiles = [
            np.where(128 * t + p <= 2 * f + h, np.float32(0.0), np.float32(MASK_NEG))
            for t in range(4)
        ]
        masks_h.append(np.concatenate(tiles, axis=1).astype(np.float32))

    xbf = x.astype(BF)
    xt_b = [np.ascontiguousarray(xbf[b].T) for b in range(B)]
    in_maps = []
    for c in range(8):
        b, h = c // 2, c % 2
        xq = np.ascontiguousarray(xbf[b, h::2, :].T)
        in_maps.append({
            "xt": xt_b[b],
            "xq": xq,
            "m": m,
            "wv": wv,
            "masks": masks_h[h],
            "ones": ones,
        })
    return in_maps


def kernel(x, W_q, W_k, W_v):
    from concourse.bass_utils import run_bass_kernel_spmd

    in_maps = build_in_maps(x, W_q, W_k, W_v)
    nc = _get_nc()
    res = run_bass_kernel_spmd(nc, in_maps, core_ids=list(range(8)))

    out = np.empty((B, S, D), dtype=np.float32)
    for c in range(8):
        b, h = c // 2, c % 2
        out[b, h::2, :] = res.results[c]["o"]
    return out
